# revision 15
# baseline (speedup 1.0000x reference)
"""Trainium2 Bass kernel for nn_BondHead2 (dense_mlp).

Computes, per batch element b (8 of them, one per NeuronCore):
    h = LN(gelu(x @ W1 + b1)); h = LN(gelu(h @ W2 + b2)); h = LN(gelu(h @ W3 + b3))
    out = h @ Wo + bo;  out = (out + out^T_{seq axes}) / 2

Design (per core):
  - feature-major layout [64 feats, T tokens], two token groups packed on 128
    partitions; fp16 activations, fp32 PSUM/stats.
  - LayerNorm mean-centering and ln_g/ln_b folded into the next layer's
    weights/biases on the host (C = I - J/64).
  - Per-token 1/std is produced via ones-matmuls (partition reduction on PE)
    stacked 16 tiles deep into PSUM banks, reshaped fat through a DRAM hop,
    rsqrt'd with a bit-trick + Newton on DVE, and broadcast back across
    partitions with a stride-0 DRAM->SBUF DMA.
  - Final 64->5 projection is stacked 6 tiles deep in PSUM; symmetrization is
    done on-device via PE transposes of 128x128 blocks.
"""

import numpy as np

import concourse.bacc as bacc
import concourse.bass as bass
import concourse.mybir as mybir
import concourse.tile as tile
from concourse.bass_utils import run_bass_kernel_spmd

F16 = mybir.dt.float16
F32 = mybir.dt.float32
U32 = mybir.dt.uint32
AF = mybir.ActivationFunctionType
OP = mybir.AluOpType

H = 64            # hidden dim
S = 384           # seq
T = S * S         # tokens per core (147456)
V = 5             # vocab
N = 512           # tokens per group-tile (free dim)
NT = T // (2 * N)  # 144 tiles (each tile = 2 groups x N tokens)
SB = 16           # tiles per stats batch
NB = NT // SB     # 9 batches
GOFF = T // 2     # token offset of group B
EPS = 1e-5
MAGIC = 0x5F3759DF

import os
NB_RUN = int(os.environ.get("K_NB", NB))      # debug: limit batches
DO_SYM = os.environ.get("K_SYM", "1") == "1"  # debug: toggle sym stage
DBG_SKIP = set(os.environ.get("K_SKIP", "").split(","))

_CACHE: dict = {}


def _build_nc():
    nc = bacc.Bacc("TRN2", target_bir_lowering=False, debug=False)

    # ---- external inputs ----
    xf = nc.dram_tensor("xf", (H, T), F16, kind="ExternalInput").ap()
    wmain = nc.dram_tensor("wmain", (128, 3 * 128), F16, kind="ExternalInput").ap()
    wstat = nc.dram_tensor("wstat", (128, SB * 32), F16, kind="ExternalInput").ap()
    wout = nc.dram_tensor("wout", (128, 3 * 30), F16, kind="ExternalInput").ap()
    biases = nc.dram_tensor("biases", (128, 3), F32, kind="ExternalInput").ap()
    id128 = nc.dram_tensor("id128", (128, 128), F32, kind="ExternalInput").ap()

    # ---- internal DRAM ----
    mobuf = nc.dram_tensor("mobuf", (V * T,), F16)          # [v, tok] linear
    r3buf = nc.dram_tensor("r3buf", (T,), F32)              # [i, j] linear
    out_vm = nc.dram_tensor("out_vm", (V * T,), F32, kind="ExternalOutput").ap()

    with tile.TileContext(nc) as tc:
        with tc.tile_pool(name="wpool", bufs=1) as wpool:
            from contextlib import ExitStack
            mstack = ExitStack()
            xpool = mstack.enter_context(tc.tile_pool(name="xpool", bufs=6))
            gpool = mstack.enter_context(tc.tile_pool(name="gpool", bufs=SB + 3))
            spool = mstack.enter_context(tc.tile_pool(name="spool", bufs=4))
            hpool = mstack.enter_context(tc.tile_pool(name="hpool", bufs=SB + 2))
            rpool = mstack.enter_context(tc.tile_pool(name="rpool", bufs=2))
            fpool = mstack.enter_context(tc.tile_pool(name="fpool", bufs=4))
            mopool = mstack.enter_context(tc.tile_pool(name="mopool", bufs=2))
            drpool = mstack.enter_context(
                tc.tile_pool(name="drpool", bufs=3, space="DRAM"))
            mps = mstack.enter_context(
                tc.tile_pool(name="mps", bufs=2, space="PSUM"))
            stps = mstack.enter_context(
                tc.tile_pool(name="stps", bufs=2, space="PSUM"))
            mops = mstack.enter_context(
                tc.tile_pool(name="mops", bufs=2, space="PSUM"))
            # resident weights
            wm = wpool.tile([128, 3 * 128], F16)
            nc.sync.dma_start(out=wm[:], in_=wmain)
            ws = wpool.tile([128, SB * 32], F16)
            nc.sync.dma_start(out=ws[:], in_=wstat)
            wo = wpool.tile([128, 3 * 30], F16)
            nc.sync.dma_start(out=wo[:], in_=wout)
            bcol = wpool.tile([128, 3], F32)
            nc.sync.dma_start(out=bcol[:], in_=biases)
            magic = wpool.tile([128, 1], U32)
            nc.vector.memset(magic[:], MAGIC)
            oneu = wpool.tile([128, 1], U32)
            nc.vector.memset(oneu[:], 1)

            def rsqrt_inplace(v_f32, scr_f32):
                """v <- rsqrt(v), elementwise, via quake seed + 2 Newton steps."""
                sh = v_f32.shape
                vi = v_f32.bitcast(U32)
                yi = scr_f32.bitcast(U32)
                # yi = MAGIC - (vi >> 1)
                nc.vector.tensor_tensor(
                    yi, vi, oneu[:].to_broadcast(sh), OP.logical_shift_right
                )
                nc.vector.tensor_tensor(
                    yi, magic[:].to_broadcast(sh), yi, OP.subtract
                )
                # now scr holds y0 (fp32). Newton: y <- y*(1.5 - 0.5*v*y^2)
                y = scr_f32
                for _ in range(2):
                    t = fpool.tile([128, 128], F32, tag="nrt")
                    nc.vector.tensor_mul(t[:], y, y)
                    nc.vector.scalar_tensor_tensor(
                        t[:], v_f32, -0.5, t[:], OP.mult, OP.mult
                    )
                    nc.vector.tensor_scalar(t[:], t[:], 1.5, None, OP.add)
                    nc.vector.tensor_mul(y, t[:], y)
                # result in scr (y); copy back to v
                nc.vector.tensor_copy(v_f32, y)

            glast = {}
            for b in range(NB_RUN):
                hcur = [None] * SB  # SBUF input tiles for current layer's MMs
                for layer in range(3):
                    statbank = stps.tile([64, N], F32, tag="stat")
                    gnew = [None] * SB
                    for p in range(SB // 2):  # pair loop
                        mpair = mps.tile([128, 2, N], F32, tag="m")
                        for k in range(2):
                            bt = 2 * p + k
                            t_glob = b * SB + bt
                            if layer == 0:
                                xt = xpool.tile([128, N], F16)
                                nc.sync.dma_start(
                                    out=xt[0:64, :],
                                    in_=xf[:, t_glob * N:(t_glob + 1) * N],
                                )
                                nc.sync.dma_start(
                                    out=xt[64:128, :],
                                    in_=xf[:, GOFF + t_glob * N:GOFF + (t_glob + 1) * N],
                                )
                                rhs = xt[:]
                            else:
                                rhs = hcur[bt][:]
                            nc.tensor.matmul(
                                mpair[:, k, :], wm[:, 128 * layer:128 * (layer + 1)],
                                rhs, start=True, stop=True,
                            )
                        g = gpool.tile([128, 2 * N], F16, tag="g")
                        nc.scalar.activation(
                            g[:], mpair[:].rearrange("p a n -> p (a n)"),
                            AF.Gelu, bias=bcol[:, layer:layer + 1], scale=1.0,
                        )
                        for k in range(2):
                            bt = 2 * p + k
                            gs = g[:, k * N:(k + 1) * N]
                            s = spool.tile([128, N], F16, tag="s")
                            nc.vector.tensor_mul(s[:], gs, gs)
                            nc.tensor.matmul(
                                statbank[0:32], ws[:, 32 * bt:32 * bt + 32], gs,
                                start=(bt == 0), stop=(bt == SB - 1),
                                skip_group_check=True,
                            )
                            nc.tensor.matmul(
                                statbank[32:64], ws[:, 32 * bt:32 * bt + 32], s[:],
                                start=(bt == 0), stop=(bt == SB - 1),
                                skip_group_check=True,
                            )
                            gnew[bt] = (g, k)
                            if layer == 2:
                                # stacked out-projection, 6 tiles per mobank
                                t_glob = b * SB + bt
                                mg = t_glob % 6
                                a, c = divmod(mg, 3)
                                if mg == 0:
                                    mob = mops.tile([128, N], F32, tag="mo")
                                    glast["mob"] = mob
                                mob = glast["mob"]
                                nc.tensor.matmul(
                                    mob[64 + 32 * a:94 + 32 * a],
                                    wo[:, 30 * c:30 * c + 30], gs,
                                    start=(c == 0), stop=(c == 2),
                                    skip_group_check=True,
                                    tile_position=(0, 64 + 32 * a),
                                )
                                if mg == 5:
                                    mocp = mopool.tile([128, N], F16, tag="mocp")
                                    nc.vector.tensor_copy(
                                        mocp[64:94], mob[64:94]
                                    )
                                    nc.vector.tensor_copy(
                                        mocp[96:126], mob[96:126]
                                    )
                                    base = (t_glob - 5) * N
                                    for a2 in range(2):
                                        for c2 in range(3):
                                            src = mocp[
                                                64 + 32 * a2 + 10 * c2:
                                                74 + 32 * a2 + 10 * c2]
                                            dst = bass.AP(
                                                tensor=mobuf,
                                                offset=base + (3 * a2 + c2) * N,
                                                ap=[[GOFF, 2], [T, V], [1, N]],
                                            )
                                            nc.sync.dma_start(out=dst, in_=src)

                    # ---- batch-layer stats -> r ----
                    rowboth = fpool.tile([64, N], F32, tag="row")
                    nc.vector.tensor_copy(rowboth[0:32], statbank[0:32])
                    nc.vector.tensor_copy(rowboth[32:64], statbank[32:64])
                    statd = drpool.tile([64 * N], F32, tag="statd")
                    nc.sync.dma_start(
                        out=statd[:].rearrange("(a n) -> a n", a=64),
                        in_=rowboth[:],
                    )
                    meanfat = fpool.tile([128, 128], F32, tag="meanfat")
                    msqfat = fpool.tile([128, 128], F32, tag="msqfat")
                    nc.sync.dma_start(
                        out=meanfat[:],
                        in_=statd[0:32 * N].rearrange("(a n) -> a n", a=128),
                    )
                    nc.sync.dma_start(
                        out=msqfat[:],
                        in_=statd[32 * N:64 * N].rearrange("(a n) -> a n", a=128),
                    )
                    # var = (msq + eps) - mean^2   (in msqfat)
                    sqf = fpool.tile([128, 128], F32, tag="sqf")
                    nc.vector.tensor_mul(sqf[:], meanfat[:], meanfat[:])
                    nc.vector.scalar_tensor_tensor(
                        msqfat[:], msqfat[:], EPS, sqf[:], OP.add, OP.subtract
                    )
                    rsqrt_inplace(msqfat[:], sqf[:])
                    if layer < 2:
                        rf16 = fpool.tile([128, 128], F16, tag="rf16")
                        nc.vector.tensor_copy(rf16[:], msqfat[:])
                        rdram = drpool.tile([64 * N], F16, tag="rdram")
                        nc.sync.dma_start(
                            out=rdram[0:32 * N].rearrange("(a n) -> a n", a=128),
                            in_=rf16[:],
                        )
                        # broadcast-replicate r rows to all partitions
                        # (group A values live at rdram[0:8192], B at [8192:16384])
                        repl = rpool.tile([128, SB * N], F16, tag="repl")
                        rd_ap = rdram[:]
                        for gidx in range(2):
                            src = bass.AP(
                                tensor=rd_ap.tensor,
                                offset=rd_ap.offset + gidx * SB * N,
                                ap=[[0, 64], [1, SB * N]],
                            )
                            nc.sync.dma_start(
                                out=repl[64 * gidx:64 * (gidx + 1), :], in_=src
                            )
                        for bt in range(SB):
                            gt, k = gnew[bt]
                            hcur_t = hpool.tile([128, N], F16, tag="h")
                            nc.vector.tensor_mul(
                                hcur_t[:], gt[:, k * N:(k + 1) * N],
                                repl[:, bt * N:(bt + 1) * N],
                            )
                            hcur[bt] = hcur_t
                    else:
                        # r3 -> DRAM, token-ordered; fat rows 0-63 = group A
                        # (row 4j+q), 64-127 = group B
                        for gidx in range(2):
                            dst = bass.AP(
                                tensor=r3buf,
                                offset=b * SB * N + gidx * GOFF,
                                ap=[[N, SB], [128, 4], [1, 128]],
                            )
                            nc.sync.dma_start(
                                out=dst, in_=msqfat[64 * gidx:64 * (gidx + 1)]
                            )

            mstack.close()
            # ---------- symmetrization ----------
            idt = wpool.tile([128, 128], F32)
            nc.sync.dma_start(out=idt[:], in_=id128)
            with (
                tc.tile_pool(name="sypool", bufs=3) as sy,
                tc.tile_pool(name="syps", bufs=3, space="PSUM") as syps,
            ):
                def load_block(bi, bj):
                    mo = sy.tile([128, V, 128], F16, tag="mo_in")
                    src = bass.AP(
                        tensor=mobuf, offset=bi * 128 * S + bj * 128,
                        ap=[[S, 128], [T, V], [1, 128]],
                    )
                    nc.sync.dma_start(out=mo[:], in_=src)
                    r = sy.tile([128, 128], F32, tag="r_in")
                    rsrc = bass.AP(
                        tensor=r3buf, offset=bi * 128 * S + bj * 128,
                        ap=[[S, 128], [1, 128]],
                    )
                    nc.sync.dma_start(out=r[:], in_=rsrc)
                    p_ = sy.tile([128, V, 128], F32, tag="p")
                    rb = bass.AP(tensor=r.tensor, offset=r.offset,
                                 ap=[r.ap[0], [0, V], r.ap[1]])
                    nc.vector.scalar_tensor_tensor(
                        p_[:], mo[:], 0.5, rb, OP.mult, OP.mult
                    )
                    return p_

                def transposes(p_):
                    tq1 = syps.tile([128, 4, 128], F32, tag="tq1")
                    tq2 = syps.tile([128, 128], F32, tag="tq2")
                    for v in range(4):
                        nc.tensor.transpose(tq1[:, v, :], p_[:, v, :], idt[:])
                    nc.tensor.transpose(tq2[:], p_[:, 4, :], idt[:])
                    return tq1, tq2

                def emit(pa, tq1, tq2, bi, bj):
                    o1 = sy.tile([128, 4, 128], F32, tag="o1")
                    o2 = sy.tile([128, 128], F32, tag="o2")
                    nc.vector.tensor_add(
                        o1[:].rearrange("p a n -> p (a n)"),
                        pa[:, 0:4, :].rearrange("p a n -> p (a n)"),
                        tq1[:].rearrange("p a n -> p (a n)"),
                    )
                    nc.vector.tensor_add(o2[:], pa[:, 4, :], tq2[:])
                    d1 = bass.AP(
                        tensor=out_vm.tensor, offset=bi * 128 * S + bj * 128,
                        ap=[[S, 128], [T, 4], [1, 128]],
                    )
                    nc.sync.dma_start(out=d1, in_=o1[:])
                    d2 = bass.AP(
                        tensor=out_vm.tensor, offset=4 * T + bi * 128 * S + bj * 128,
                        ap=[[S, 128], [1, 128]],
                    )
                    nc.sync.dma_start(out=d2, in_=o2[:])

                for bi in range(3 if DO_SYM else 0):
                    for bj in range(bi + 1):
                        pa = load_block(bi, bj)
                        if bi == bj:
                            tq1, tq2 = transposes(pa)
                            emit(pa, tq1, tq2, bi, bj)
                        else:
                            pb = load_block(bj, bi)
                            tqb1, tqb2 = transposes(pb)
                            emit(pa, tqb1, tqb2, bi, bj)
                            tqa1, tqa2 = transposes(pa)
                            emit(pb, tqa1, tqa2, bj, bi)

    nc.compile()
    return nc


def _prep_weights(inputs):
    W1 = np.asarray(inputs["W1"], np.float64)
    W2 = np.asarray(inputs["W2"], np.float64)
    W3 = np.asarray(inputs["W3"], np.float64)
    Wo = np.asarray(inputs["Wo"], np.float64)
    b1 = np.asarray(inputs["b1"], np.float64)
    b2 = np.asarray(inputs["b2"], np.float64)
    b3 = np.asarray(inputs["b3"], np.float64)
    bo = np.asarray(inputs["bo"], np.float64)
    ln_g = np.asarray(inputs["ln_g"], np.float64)
    ln_b = np.asarray(inputs["ln_b"], np.float64)

    C = np.eye(H) - np.ones((H, H)) / H
    F = C @ np.diag(ln_g)
    Ws = [W1, F @ W2, F @ W3]
    bs = [b1, b2 + W2.T @ ln_b, b3 + W3.T @ ln_b]
    Woh = F @ Wo
    boh = (bo + Wo.T @ ln_b).astype(np.float32)

    wmain = np.zeros((128, 3 * 128), np.float16)
    for l, W in enumerate(Ws):
        wmain[0:64, 128 * l:128 * l + 64] = W.astype(np.float16)
        wmain[64:128, 128 * l + 64:128 * l + 128] = W.astype(np.float16)
    # stats lhsT variant bt: rows 0-15 of the PSUM bank hold group-A means
    # (row bt), rows 16-31 group-B means (row SB+bt)
    wstat = np.zeros((128, SB * 32), np.float16)
    for bt in range(SB):
        wstat[0:64, 32 * bt + bt] = np.float16(1 / 64)
        wstat[64:128, 32 * bt + SB + bt] = np.float16(1 / 64)
    wout = np.zeros((128, 3 * 30), np.float16)
    for c in range(3):
        wout[0:64, 30 * c + 10 * c:30 * c + 10 * c + V] = Woh.astype(np.float16)
        wout[64:128, 30 * c + 10 * c + V:30 * c + 10 * c + 10] = Woh.astype(np.float16)
    biases = np.zeros((128, 3), np.float32)
    for l, bb in enumerate(bs):
        biases[0:64, l] = bb.astype(np.float32)
        biases[64:128, l] = bb.astype(np.float32)
    id128 = np.eye(128, dtype=np.float32)
    return dict(wmain=wmain, wstat=wstat, wout=wout, biases=biases,
                id128=id128), boh


def kernel(**inputs):
    if "nc" not in _CACHE:
        _CACHE["nc"] = _build_nc()
    nc = _CACHE["nc"]
    weights, boh = _prep_weights(inputs)

    x = np.asarray(inputs["x"])  # [8, S, S, H] fp32
    in_maps = []
    for b in range(8):
        xf = np.ascontiguousarray(x[b].reshape(T, H).T).astype(np.float16)
        m = dict(weights)
        m["xf"] = xf
        in_maps.append(m)

    res = run_bass_kernel_spmd(nc, in_maps, core_ids=list(range(8)))
    outs = []
    for b in range(8):
        vm = res.results[b]["out_vm"].reshape(V, S, S)
        outs.append(vm.transpose(1, 2, 0) + boh[None, None, :])
    return np.stack(outs).astype(np.float32)
